# revision 1
# baseline (speedup 1.0000x reference)
"""Trainium2 Bass kernel for nn_AttnPool_73409581023420.

Reference computation (N=64, T=256, D=768, H=256, M=N*T=16384):
    xf = x.reshape(M, D)
    q, k, v = xf @ Wq.T, xf @ Wk.T, xf @ Wv.T
    att = softmax(q @ k.T / sqrt(H))            # [M, M]
    out = ((att @ v) @ Wo.T).mean(0)            # [1, D]

Key identity: only the column-sums of `att` matter for the mean:
    out = (colsum(att) @ xf) @ Wv.T @ Wo.T / M
so V is never materialized and att@v is never computed. The kernel
computes, per core c (2048 query rows each):
    s_c[j] = sum_{i in shard_c} exp(q_i.k_j/16) / Z_i     for all j in [M]
on device (projections + 16384x16384/8 scores + softmax colsum), and the
host finishes with s = sum_c s_c, then the tiny [1,768] epilogue.

Device layout per core (HW exec ~376-388us, ScalarE-exp-bound steady state):
  - inputs (host pre-transposed/cast): xT [768,2048] bf16, wqT/wkT [768,256] bf16
  - Q^T,K^T projected on TensorE in bf16, stored fp8 e4m3 as [128, 2, cols]
    (head-dim split) so one DoubleRow matmul contracts all 256 head dims
  - K^T all-gathered across the 8 cores (AllGather, 512KB/rank fp8)
  - per 128-row q-block: fp8 DoubleRow scores matmuls (fp32 PSUM, 2048-col
    chunks, double-buffered 4-bank tiles) -> ScalarE exp(scale=1/16) with
    fused row-sum accum_out (softmax Z for free) -> bf16 E
    -> VectorE tensor_scalar_mul (4x mode) + tensor_add (2x mode):
       acc += E * (1/Z)   [scalar_tensor_tensor only has a 1x uop]
  - final collapse of acc's 128 partitions via one-hot-windowed TensorE
    matmuls (PE out base partition is limited to {0,32,64}, so one-hot lhsT
    columns route j-tile t to PSUM row t%4); the last q-block's weighted
    colsum folds into the same accumulation groups via a w-valued window,
    keeping the tail off VectorE -> s_out [n_pass, 32, 2048] f32
"""

import numpy as np
import ml_dtypes

# Full-size problem constants (hardcoded per contract; kernel.py may not
# read spec/reference files).
N_CORES = 8
M_TOTAL = 16384          # N*T
D_MODEL = 768
H_DIM = 256
ROWS_PER_CORE = M_TOTAL // N_CORES   # 2048
SCALE = 1.0 / 16.0       # 1/sqrt(H)

_BF16 = ml_dtypes.bfloat16

_PROGRAM_CACHE = {}


def build_program(n_cores=N_CORES, rows_per_core=ROWS_PER_CORE, d_model=D_MODEL,
                  h_dim=H_DIM, scale=SCALE):
    """Build + compile the SPMD Bass program. Returns the compiled Bacc."""
    import concourse.bass as bass
    import concourse.mybir as mybir
    import concourse.tile as tile
    from concourse import bacc

    f32 = mybir.dt.float32
    bf16 = mybir.dt.bfloat16
    f8 = mybir.dt.float8e4

    P = 128                       # partitions
    JT = 512                      # j-tile (matmul moving free dim / psum bank)
    m_total = n_cores * rows_per_core
    n_qb = rows_per_core // P     # q-blocks per core
    n_hb = h_dim // P             # head-dim blocks (contract chunks for scores)
    n_dc = d_model // P           # contract chunks for projections
    it_jt = min(JT, rows_per_core)  # projection i-tile width
    n_it = rows_per_core // it_jt   # i-tiles per projection pass
    chunk = min(2048, m_total)    # score/exp chunk (<= 4 psum banks)
    n_ch = m_total // chunk       # chunks per q-block
    jt_per_chunk = chunk // JT
    n_jt = m_total // JT          # j-tiles total
    tiles_per_pass = 16           # collapse: 4 banks x 4 partition-rows
    n_pass = (n_jt + tiles_per_pass - 1) // tiles_per_pass

    nc = bacc.Bacc("TRN2", target_bir_lowering=False, debug=False,
                   num_devices=n_cores)

    xT = nc.dram_tensor("xT", [d_model, rows_per_core], bf16, kind="ExternalInput")
    wqT = nc.dram_tensor("wqT", [d_model, h_dim], bf16, kind="ExternalInput")
    wkT = nc.dram_tensor("wkT", [d_model, h_dim], bf16, kind="ExternalInput")
    s_out = nc.dram_tensor("s_out", [n_pass, 32, chunk], f32, kind="ExternalOutput")
    kt_bounce = nc.dram_tensor("kt_bounce", [n_hb, P, rows_per_core],
                               mybir.dt.float8e4, kind="Internal")
    kt_gather = nc.dram_tensor("kt_gather", [n_cores, n_hb, P, rows_per_core],
                               mybir.dt.float8e4, kind="Internal",
                               addr_space="Shared" if n_cores > 4 else "Local")

    xT_ap = xT.ap()
    gather_ap = kt_gather.ap()
    bounce_ap = kt_bounce.ap()
    s_out_ap = s_out.ap()

    with tile.TileContext(nc) as tc:
        with tc.tile_pool(name="persist", bufs=1) as persist, \
             tc.tile_pool(name="weights", bufs=1) as wpool, \
             tc.tile_pool(name="xstream", bufs=4) as xtp, \
             tc.tile_pool(name="evolve", bufs=2) as epool, \
             tc.tile_pool(name="stats", bufs=2) as spool, \
             tc.tile_pool(name="sout", bufs=2) as opool:

            ones = persist.tile([P, 1], bf16, tag="ones")
            nc.vector.memset(ones[:], 1.0)
            # touch Exp early so the ~2.7us ACT table load runs during the
            # projection/all-gather phase, not before the first real exp
            scratch = persist.tile([P, 1], f32, tag="scratch")
            nc.scalar.activation(out=scratch[:], in_=ones[:],
                                 func=mybir.ActivationFunctionType.Exp)
            # one-hot window buffer: oh[:, 31-r:63-r] has column r == 1
            oh = persist.tile([P, 64], bf16, tag="oh")
            nc.vector.memset(oh[:], 0.0)
            nc.vector.memset(oh[:, 31:32], 1.0)
            # w-window: wwin[:, 127-r:255-r] has column r == w (for the last
            # q-block, whose weighted colsum folds into the collapse matmuls)
            wwin = persist.tile([P, 64], bf16, tag="wwin")
            nc.vector.memset(wwin[:], 0.0)

            # K^T/Q^T in fp8 e4m3 with the head-dim split [P, n_hb, cols]
            # so a single DoubleRow matmul contracts all 256 head dims.
            kt_full = persist.tile([P, n_hb, m_total], f8, tag="ktf")
            qt = persist.tile([P, n_hb, rows_per_core], f8, tag="qt")
            kt_loc = persist.tile([P, n_hb, rows_per_core], f8, tag="ktl")
            acc = persist.tile([P, m_total], bf16, tag="acc")
            tmp = persist.tile([P, min(4096, m_total)], bf16, tag="tmp")

            wq_sb = wpool.tile([P, n_dc * h_dim], bf16, tag="wq")
            wk_sb = wpool.tile([P, n_dc * h_dim], bf16, tag="wk")
            for ch in range(n_dc):
                # split triggers across two idle queues (they serialize at
                # ~0.6us each per engine and pace the projection phase)
                nc.sync.dma_start(out=wk_sb[:, ch * h_dim:(ch + 1) * h_dim],
                                  in_=wkT.ap()[ch * P:(ch + 1) * P, :])
                nc.scalar.dma_start(out=wq_sb[:, ch * h_dim:(ch + 1) * h_dim],
                                    in_=wqT.ap()[ch * P:(ch + 1) * P, :])

            def projection(w_sb, dst_tiles, tagp):
                # dst[:, hb, it*JT:...] = (W x^T)[hb*P:(hb+1)*P, i-tile]
                with tc.tile_pool(name=f"pp_{tagp}", bufs=2, space="PSUM") as pp:
                    for it in range(n_it):
                        pss = [pp.tile([P, it_jt], f32, tag=f"ps{hb}", name=f"ps{hb}")
                               for hb in range(n_hb)]
                        for ch in range(n_dc):
                            xt = xtp.tile([P, it_jt], bf16, tag="xt")
                            eng = nc.sync if ch % 2 == 0 else nc.scalar
                            eng.dma_start(
                                out=xt[:],
                                in_=xT_ap[ch * P:(ch + 1) * P,
                                          it * it_jt:(it + 1) * it_jt])
                            for hb in range(n_hb):
                                nc.tensor.matmul(
                                    pss[hb][:],
                                    lhsT=w_sb[:, ch * h_dim + hb * P:
                                              ch * h_dim + (hb + 1) * P],
                                    rhs=xt[:],
                                    start=(ch == 0), stop=(ch == n_dc - 1))
                        for hb in range(n_hb):
                            nc.vector.tensor_copy(
                                dst_tiles[:, hb,
                                          it * it_jt:(it + 1) * it_jt],
                                pss[hb][:])

            # K first so the all-gather starts early; Q overlaps the gather.
            projection(wk_sb, kt_loc, "k")
            for hb in range(n_hb):
                nc.sync.dma_start(out=bounce_ap[hb], in_=kt_loc[:, hb, :])
            if n_cores > 1:
                nc.gpsimd.collective_compute(
                    "AllGather",
                    mybir.AluOpType.bypass,
                    replica_groups=[list(range(n_cores))],
                    ins=[bounce_ap],
                    outs=[gather_ap],
                )
            projection(wq_sb, qt, "q")

            for r in range(n_cores):
                for hb in range(n_hb):
                    if n_cores > 1:
                        srcap = gather_ap[r, hb]
                    else:
                        srcap = bounce_ap[hb]
                    nc.sync.dma_start(
                        out=kt_full[:, hb, r * rows_per_core:
                                    (r + 1) * rows_per_core],
                        in_=srcap)

            with tc.tile_pool(name="psc", bufs=2, space="PSUM") as psc:
                for qb in range(n_qb):
                    E = epool.tile([P, m_total], bf16, tag="E")
                    zp = spool.tile([P, n_ch], f32, tag="zp")
                    for ck in range(n_ch):
                        ps = psc.tile([P, chunk], f32, tag="ps")
                        for jt in range(jt_per_chunk):
                            j0 = ck * chunk + jt * JT
                            nc.tensor.matmul(
                                ps[:, jt * JT:(jt + 1) * JT],
                                lhsT=qt[:, :, qb * P:(qb + 1) * P],
                                rhs=kt_full[:, :, j0:j0 + JT],
                                perf_mode=mybir.MatmulPerfMode.DoubleRow,
                                start=True, stop=True)
                        nc.scalar.activation(
                            out=E[:, ck * chunk:(ck + 1) * chunk],
                            in_=ps[:],
                            func=mybir.ActivationFunctionType.Exp,
                            scale=scale,
                            accum_out=zp[:, ck:ck + 1])
                    z = spool.tile([P, 1], f32, tag="z")
                    if n_ch > 1:
                        nc.vector.reduce_sum(z[:], zp[:], axis=mybir.AxisListType.X)
                    else:
                        nc.vector.tensor_copy(z[:], zp[:])
                    w = spool.tile([P, 1], f32, tag="w")
                    nc.vector.reciprocal(w[:], z[:])
                    if qb == n_qb - 1 and n_qb > 1:
                        # last block: fold E*w into the collapse matmuls on
                        # TensorE (idle at the tail) instead of DVE
                        nc.vector.tensor_copy(wwin[:, 31:32], w[:])
                        E_last = E
                        continue
                    # acc += E * w in quarter slices. scalar_tensor_tensor
                    # only has a 1x uop; tensor_scalar (4x) + tensor_tensor
                    # add (2x_1P) is ~25% faster on DVE.
                    qr = min(4096, m_total)
                    for qtr in range(m_total // qr):
                        lo, hi = qtr * qr, (qtr + 1) * qr
                        if qb == 0:
                            nc.vector.tensor_scalar_mul(
                                acc[:, lo:hi], E[:, lo:hi], w[:])
                        else:
                            nc.vector.tensor_scalar_mul(tmp[:], E[:, lo:hi], w[:])
                            nc.vector.tensor_add(acc[:, lo:hi],
                                                 acc[:, lo:hi], tmp[:])

                # Collapse acc's 128 partitions: for each bank, the 4
                # j-tiles accumulate into the same [32, JT] PSUM region with
                # one-hot lhsT columns routing tile r to partition row r.
                # Half-chunk passes interleave with the last q-block's
                # accumulate quarters.
                half_tpp = tiles_per_pass // 2
                for p2 in range(2 * n_pass):
                    p, lohalf = p2 // 2, p2 % 2
                    ntt0 = min(tiles_per_pass, n_jt - p * tiles_per_pass)
                    ntt = (min(ntt0, half_tpp) if lohalf == 0
                           else max(0, ntt0 - half_tpp))
                    if ntt <= 0:
                        continue
                    cps = psc.tile([P, chunk // 2], f32, tag="ps")
                    fold_last = n_qb > 1
                    for b in range((ntt + 3) // 4):
                        nr = min(4, ntt - 4 * b)
                        for r in range(nr):
                            t = (p * tiles_per_pass + lohalf * half_tpp
                                 + 4 * b + r)
                            nc.tensor.matmul(
                                cps[0:32, b * JT:(b + 1) * JT],
                                lhsT=oh[:, 31 - r:63 - r],
                                rhs=acc[:, t * JT:(t + 1) * JT],
                                start=(r == 0),
                                stop=(r == nr - 1 and not fold_last))
                        if fold_last:
                            for r in range(nr):
                                t = (p * tiles_per_pass + lohalf * half_tpp
                                     + 4 * b + r)
                                nc.tensor.matmul(
                                    cps[0:32, b * JT:(b + 1) * JT],
                                    lhsT=wwin[:, 31 - r:63 - r],
                                    rhs=E_last[:, t * JT:(t + 1) * JT],
                                    start=False, stop=(r == nr - 1))
                    ncol = ((ntt + 3) // 4) * JT
                    sb = opool.tile([32, chunk // 2], f32, tag="sb")
                    nc.vector.tensor_copy(sb[:, :ncol], cps[0:32, :ncol])
                    nc.sync.dma_start(
                        out=s_out_ap[p][:, lohalf * (chunk // 2):
                                        lohalf * (chunk // 2) + ncol],
                        in_=sb[:, :ncol])

    nc.compile()
    return nc


def _get_program():
    key = "full"
    if key not in _PROGRAM_CACHE:
        _PROGRAM_CACHE[key] = build_program()
    return _PROGRAM_CACHE[key]


def decode_s(s_out_np, n_jt=M_TOTAL // 512, chunk=2048):
    """Map s_out [n_pass,32,chunk] back to the flat colsum vector."""
    jt = 512
    tiles_per_pass = 16
    s = np.zeros(n_jt * jt, np.float32)
    for p in range(s_out_np.shape[0]):
        ntt = min(tiles_per_pass, n_jt - p * tiles_per_pass)
        for tt in range(ntt):
            t = p * tiles_per_pass + tt
            b, r = tt // 4, tt % 4
            s[t * jt:(t + 1) * jt] = s_out_np[p, r, b * jt:(b + 1) * jt]
    return s


def shard_inputs(x, Wq, Wk):
    """Host-side sharding: pre-transpose + cast to bf16 per core."""
    xf = np.ascontiguousarray(x, dtype=np.float32).reshape(M_TOTAL, D_MODEL)
    wqT = np.ascontiguousarray(Wq.T).astype(_BF16)
    wkT = np.ascontiguousarray(Wk.T).astype(_BF16)
    in_maps = []
    for c in range(N_CORES):
        sh = xf[c * ROWS_PER_CORE:(c + 1) * ROWS_PER_CORE]
        in_maps.append({
            "xT": np.ascontiguousarray(sh.T).astype(_BF16),
            "wqT": wqT,
            "wkT": wkT,
        })
    return xf, in_maps


def run_device(nc, in_maps, trace=False, **kwargs):
    from concourse import bass_utils
    return bass_utils.run_bass_kernel_spmd(
        nc, in_maps, core_ids=list(range(len(in_maps))), trace=trace, **kwargs)


def kernel(x, Wq, Wk, Wv, Wo):
    x = np.asarray(x)
    nc = _get_program()
    xf, in_maps = shard_inputs(x, np.asarray(Wq), np.asarray(Wk))
    res = run_device(nc, in_maps)
    s = np.zeros(M_TOTAL, np.float32)
    for c in range(N_CORES):
        s += decode_s(res.results[c]["s_out"])
    y = s.astype(np.float32) @ xf                      # [D]
    pooled = (y @ np.asarray(Wv, np.float32).T) @ np.asarray(Wo, np.float32).T
    return (pooled / np.float32(M_TOTAL)).reshape(1, D_MODEL).astype(np.float32)

